# revision 12
# baseline (speedup 1.0000x reference)
"""CWICLinear eval-mode forward on 8 Trainium2 NeuronCores.

Sharding: tensor-parallel over stripes — core c owns stripes {2c, 2c+1},
i.e. output columns [1024*c, 1024*(c+1)).  x is replicated (transposed on
host so the contraction dim lands on SBUF partitions); weight/thresholds/
bias are sliced per core.  Each core computes its y^T slice plus the
per-token active-feature counts for its two stripes; the host concatenates
y slices and sums the counts.

Numerics: the gate mask |x - med| > thr is computed bit-exactly in fp32
(GPSIMD/ACT subtract and Abs are exact, DVE is_gt is an exact fp32
compare), so active_params matches the reference exactly.  The matmul
runs in bf16 with fp32 PSUM accumulation (~3e-3 max rel err on y).
"""
import numpy as np
import ml_dtypes

import concourse.bacc as bacc_mod
import concourse.tile as tile
from concourse import mybir
from concourse.bass_utils import run_bass_kernel_spmd

# Problem constants (hardcoded per harness contract).
IN_F = 2048          # I: input features (contraction dim)
TOK = 2048           # T: batch*seq tokens
OUT_F = 8192         # total output features
STRIPE = 512
N_STRIPES = 16
N_CORES = 8
O_CORE = OUT_F // N_CORES        # 1024 output cols per core
NS_CORE = N_STRIPES // N_CORES   # 2 stripes per core
KC = IN_F // 128                 # 16 contraction chunks
TBLK = 512                       # token block per outer iteration
NTB = TOK // TBLK                # 4 token blocks
THRESH_LR_SCALE = np.float32(IN_F ** 0.5)

_BF16 = mybir.dt.bfloat16
_F32 = mybir.dt.float32


def _build():
    nc = bacc_mod.Bacc("TRN2")
    xT = nc.dram_tensor("xT", [IN_F, TOK], _F32, kind="ExternalInput")
    wb = nc.dram_tensor("wb", [IN_F, O_CORE], _BF16, kind="ExternalInput")
    negmed = nc.dram_tensor("negmed", [KC, 128], _F32, kind="ExternalInput")
    thr = nc.dram_tensor("thr", [NS_CORE, KC, 128], _F32, kind="ExternalInput")
    pmb = nc.dram_tensor("pmb", [O_CORE // 128, 128], _F32, kind="ExternalInput")

    yT = nc.dram_tensor("yT", [O_CORE, TOK], _F32, kind="ExternalOutput")
    cnt = nc.dram_tensor("cnt", [1, TOK], _F32, kind="ExternalOutput")

    with tile.TileContext(nc) as tc:
        with tc.tile_pool(name="const", bufs=1) as constp, \
             tc.tile_pool(name="wp", bufs=1) as wp, \
             tc.tile_pool(name="xin", bufs=3) as xin, \
             tc.tile_pool(name="elem", bufs=2) as elem, \
             tc.tile_pool(name="xmp", bufs=2) as xmp, \
             tc.tile_pool(name="yout", bufs=4) as yout, \
             tc.tile_pool(name="ps", bufs=7, space="PSUM") as ps, \
             tc.tile_pool(name="psc", bufs=1, space="PSUM") as psc:

            # ---- persistent constants ----
            negmed_sb = constp.tile([128, KC], _F32)
            nc.gpsimd.dma_start(out=negmed_sb,
                                in_=negmed[:, :].rearrange("k p -> p k"))
            thr_sb = []
            for n in range(NS_CORE):
                t = constp.tile([128, KC], _F32, tag=f"thr{n}")
                nc.gpsimd.dma_start(out=t, in_=thr[n].rearrange("k p -> p k"))
                thr_sb.append(t)
            pmb_sb = constp.tile([128, O_CORE // 128], _F32)
            nc.gpsimd.dma_start(out=pmb_sb,
                                in_=pmb[:, :].rearrange("o p -> p o"))
            ones_sb = constp.tile([128, 1], _BF16)
            nc.vector.memset(ones_sb, 1.0)

            # PE warm-up: ~4us of tiny matmuls during the otherwise-idle
            # head so the HAM clock gate reaches 8/8 before real work.
            # They write cnt psum [0,0], which the real count chain's
            # start=True overwrites.
            warm_cps = psc.tile([1, TBLK], _F32, tag="cps")
            for _ in range(140):
                nc.tensor.matmul(warm_cps[0:1, 0:1], ones_sb, ones_sb,
                                 start=True, stop=True)

            # ---- resident weights (bf16, cast on host), loaded interleaved
            # with the first token block's x tiles so PE starts early ----
            w_sb = [None] * KC

            for tb in range(NTB):
                tsl = slice(tb * TBLK, (tb + 1) * TBLK)
                xms = [[None] * KC for _ in range(NS_CORE)]
                cps = psc.tile([1, TBLK], _F32, tag="cps")
                for k in range(KC):
                    xt = xin.tile([128, TBLK], _F32, tag="xt")
                    nc.sync.dma_start(out=xt, in_=xT[k * 128:(k + 1) * 128, tsl])
                    if tb == 0:
                        t = wp.tile([128, O_CORE], _BF16, tag=f"w{k}")
                        nc.sync.dma_start(out=t, in_=wb[k * 128:(k + 1) * 128, :])
                        w_sb[k] = t
                    # exact xd = x - med on GPSIMD; exact |xd| on ACT
                    xd = elem.tile([128, TBLK], _F32, tag="xd")
                    nc.gpsimd.tensor_scalar_add(xd, xt, negmed_sb[:, k:k + 1])
                    axd = elem.tile([128, TBLK], _F32, tag="axd")
                    nc.scalar.activation(axd, xt, mybir.ActivationFunctionType.Abs,
                                         bias=negmed_sb[:, k:k + 1], scale=1.0)
                    xdb = elem.tile([128, TBLK], _BF16, tag="xdb")
                    nc.scalar.copy(xdb, xd)
                    masks = []
                    for n in range(NS_CORE):
                        mk = elem.tile([128, TBLK], _BF16, tag=f"mk{n}")
                        nc.vector.tensor_scalar(mk, axd, thr_sb[n][:, k:k + 1],
                                                None, op0=mybir.AluOpType.is_gt)
                        masks.append(mk)
                        xm = xmp.tile([128, TBLK], _BF16, tag=f"xm{n}_{k}")
                        nc.vector.tensor_tensor(out=xm, in0=xdb, in1=mk,
                                                op=mybir.AluOpType.mult)
                        xms[n][k] = xm
                    # mask-sum tree: per-chunk on GPSIMD, pair+quad on DVE
                    # (values <= 8, exact in bf16) -> 4 count matmuls per block
                    ms = elem.tile([128, TBLK], _BF16, tag="ms", bufs=3)
                    nc.gpsimd.tensor_tensor(out=ms, in0=masks[0], in1=masks[1],
                                            op=mybir.AluOpType.add)
                    if k % 2 == 0:
                        ms_prev = ms
                    else:
                        ms2 = elem.tile([128, TBLK], _BF16, tag="ms2", bufs=3)
                        nc.vector.tensor_tensor(out=ms2, in0=ms_prev, in1=ms,
                                                op=mybir.AluOpType.add)
                        if (k // 2) % 2 == 0:
                            ms2_prev = ms2
                        else:
                            ms4 = elem.tile([128, TBLK], _BF16, tag="ms4", bufs=3)
                            nc.vector.tensor_tensor(out=ms4, in0=ms2_prev, in1=ms2,
                                                    op=mybir.AluOpType.add)
                            nc.tensor.matmul(cps, ones_sb, ms4,
                                             start=(k == 3), stop=(k == KC - 1))
                csb = yout.tile([1, TBLK], _F32, tag="csb")
                nc.scalar.copy(csb, cps)
                nc.sync.dma_start(out=cnt[0:1, tsl], in_=csb)

                # y matmuls: y^T[o_chunk, t] accumulated over k
                for n in range(NS_CORE):
                    for o in range(STRIPE // 128):
                        col0 = n * STRIPE + o * 128
                        yps = ps.tile([128, TBLK], _F32, tag="yps")
                        for k in range(KC):
                            nc.tensor.matmul(
                                yps,
                                w_sb[k][:, col0:col0 + 128],
                                xms[n][k],
                                start=(k == 0), stop=(k == KC - 1))
                        ysb = yout.tile([128, TBLK], _F32, tag="ysb")
                        oc = col0 // 128
                        nc.scalar.activation(
                            ysb, yps, mybir.ActivationFunctionType.Identity,
                            bias=pmb_sb[:, oc:oc + 1], scale=1.0)
                        nc.gpsimd.dma_start(out=yT[col0:col0 + 128, tsl], in_=ysb)
    nc.finalize()
    return nc


_NC_CACHE = None


def _get_nc():
    global _NC_CACHE
    if _NC_CACHE is None:
        _NC_CACHE = _build()
    return _NC_CACHE


def _prep_in_maps(x, weight, bias, thresholds, adj_med, adj_std):
    x = np.asarray(x, dtype=np.float32)
    weight = np.asarray(weight, dtype=np.float32)
    bias = np.asarray(bias, dtype=np.float32)
    thresholds = np.asarray(thresholds, dtype=np.float32)
    adj_med = np.asarray(adj_med, dtype=np.float32)
    adj_std = np.asarray(adj_std, dtype=np.float32)

    xT = np.ascontiguousarray(x.reshape(TOK, IN_F).T)
    # effective thresholds, matching the reference's op order exactly:
    # (thresholds * adj_std) * SCALE, all fp32 elementwise
    thr_eff = (thresholds * adj_std[None, :]) * THRESH_LR_SCALE
    negmed = np.ascontiguousarray((-adj_med).reshape(KC, 128))
    post_mu_bias = (adj_med @ weight + bias).astype(np.float32)

    in_maps = []
    for c in range(N_CORES):
        cols = slice(c * O_CORE, (c + 1) * O_CORE)
        wb = np.ascontiguousarray(weight[:, cols]).astype(ml_dtypes.bfloat16)
        thr_c = np.ascontiguousarray(
            thr_eff[c * NS_CORE:(c + 1) * NS_CORE].reshape(NS_CORE, KC, 128))
        pmb_c = np.ascontiguousarray(post_mu_bias[cols].reshape(O_CORE // 128, 128))
        in_maps.append({"xT": xT, "wb": wb, "negmed": negmed,
                        "thr": thr_c, "pmb": pmb_c})
    return in_maps


def _run(inputs, trace=False, trace_kwargs=None):
    nc = _get_nc()
    in_maps = _prep_in_maps(**inputs)
    kw = {}
    if trace:
        kw["trace"] = True
        if trace_kwargs:
            kw.update(trace_kwargs)
    return run_bass_kernel_spmd(nc, in_maps, core_ids=list(range(N_CORES)), **kw)


def _assemble(results, x_dtype=np.float32):
    y = np.empty((TOK, OUT_F), dtype=np.float32)
    cnt_total = np.zeros(TOK, dtype=np.float32)
    for c, r in enumerate(results):
        y[:, c * O_CORE:(c + 1) * O_CORE] = r["yT"].T
        cnt_total += r["cnt"][0]
    y = y.reshape(1, TOK, OUT_F)
    active = (np.float32(STRIPE) * cnt_total).reshape(1, TOK)
    dense = np.full((1, TOK), np.float32(STRIPE * N_STRIPES * IN_F),
                    dtype=np.float32)
    return y, (dense, active)


def kernel(x, weight, bias, thresholds, adj_med, adj_std):
    res = _run(dict(x=x, weight=weight, bias=bias, thresholds=thresholds,
                    adj_med=adj_med, adj_std=adj_std))
    return _assemble(res.results)


# revision 14
# speedup vs baseline: 1.0056x; 1.0056x over previous
"""CWICLinear eval-mode forward on 8 Trainium2 NeuronCores.

Sharding: tensor-parallel over stripes — core c owns stripes {2c, 2c+1},
i.e. output columns [1024*c, 1024*(c+1)).  x is replicated (transposed on
host so the contraction dim lands on SBUF partitions); weight/thresholds/
bias are sliced per core.  Each core computes its y^T slice plus the
per-token active-feature counts for its two stripes; the host concatenates
y slices and sums the counts.

Numerics: the gate mask |x - med| > thr is computed bit-exactly in fp32
(GPSIMD/ACT subtract and Abs are exact, DVE is_gt is an exact fp32
compare), so active_params matches the reference exactly.  The matmul
runs in bf16 with fp32 PSUM accumulation (~3e-3 max rel err on y).
"""
import numpy as np
import ml_dtypes

import concourse.bacc as bacc_mod
import concourse.tile as tile
from concourse import mybir
from concourse.bass_utils import run_bass_kernel_spmd

# Problem constants (hardcoded per harness contract).
IN_F = 2048          # I: input features (contraction dim)
TOK = 2048           # T: batch*seq tokens
OUT_F = 8192         # total output features
STRIPE = 512
N_STRIPES = 16
N_CORES = 8
O_CORE = OUT_F // N_CORES        # 1024 output cols per core
NS_CORE = N_STRIPES // N_CORES   # 2 stripes per core
KC = IN_F // 128                 # 16 contraction chunks
TBLK = 512                       # token block per outer iteration
NTB = TOK // TBLK                # 4 token blocks
THRESH_LR_SCALE = np.float32(IN_F ** 0.5)

_BF16 = mybir.dt.bfloat16
_F32 = mybir.dt.float32


def _build():
    nc = bacc_mod.Bacc("TRN2")
    xT = nc.dram_tensor("xT", [IN_F, TOK], _F32, kind="ExternalInput")
    wb = nc.dram_tensor("wb", [IN_F, O_CORE], _BF16, kind="ExternalInput")
    negmed = nc.dram_tensor("negmed", [KC, 128], _F32, kind="ExternalInput")
    thr = nc.dram_tensor("thr", [NS_CORE, KC, 128], _F32, kind="ExternalInput")
    pmb = nc.dram_tensor("pmb", [O_CORE // 128, 128], _F32, kind="ExternalInput")

    yT = nc.dram_tensor("yT", [O_CORE, TOK], _F32, kind="ExternalOutput")
    cnt = nc.dram_tensor("cnt", [1, TOK], _F32, kind="ExternalOutput")

    with tile.TileContext(nc) as tc:
        with tc.tile_pool(name="const", bufs=1) as constp, \
             tc.tile_pool(name="wp", bufs=1) as wp, \
             tc.tile_pool(name="xin", bufs=3) as xin, \
             tc.tile_pool(name="elem", bufs=2) as elem, \
             tc.tile_pool(name="xmp", bufs=2) as xmp, \
             tc.tile_pool(name="yout", bufs=4) as yout, \
             tc.tile_pool(name="ps", bufs=7, space="PSUM") as ps, \
             tc.tile_pool(name="psc", bufs=1, space="PSUM") as psc:

            # ---- persistent constants ----
            negmed_sb = constp.tile([128, KC], _F32)
            nc.gpsimd.dma_start(out=negmed_sb,
                                in_=negmed[:, :].rearrange("k p -> p k"))
            thr_sb = []
            for n in range(NS_CORE):
                t = constp.tile([128, KC], _F32, tag=f"thr{n}")
                nc.gpsimd.dma_start(out=t, in_=thr[n].rearrange("k p -> p k"))
                thr_sb.append(t)
            pmb_sb = constp.tile([128, O_CORE // 128], _F32)
            nc.gpsimd.dma_start(out=pmb_sb,
                                in_=pmb[:, :].rearrange("o p -> p o"))
            ones_sb = constp.tile([128, 1], _BF16)
            nc.vector.memset(ones_sb, 1.0)

            # PE warm-up: ~4us of tiny matmuls during the otherwise-idle
            # head so the HAM clock gate reaches 8/8 before real work.
            # They write cnt psum [0,0], which the real count chain's
            # start=True overwrites.
            warm_cps = psc.tile([1, TBLK], _F32, tag="cps")
            for _ in range(140):
                nc.tensor.matmul(warm_cps[0:1, 0:1], ones_sb, ones_sb,
                                 start=True, stop=True)

            # ---- resident weights (bf16, cast on host), loaded interleaved
            # with the first token block's x tiles so PE starts early ----
            w_sb = [None] * KC

            for tb in range(NTB):
                tsl = slice(tb * TBLK, (tb + 1) * TBLK)
                xms = [[None] * KC for _ in range(NS_CORE)]
                cps = psc.tile([1, TBLK], _F32, tag="cps")
                for k in range(KC):
                    xt = xin.tile([128, TBLK], _F32, tag="xt")
                    nc.sync.dma_start(out=xt, in_=xT[k * 128:(k + 1) * 128, tsl])
                    if tb == 0:
                        t = wp.tile([128, O_CORE], _BF16, tag=f"w{k}")
                        nc.sync.dma_start(out=t, in_=wb[k * 128:(k + 1) * 128, :])
                        w_sb[k] = t
                    # exact xd = x - med on GPSIMD; exact |xd| on ACT
                    xd = elem.tile([128, TBLK], _F32, tag="xd")
                    nc.gpsimd.tensor_scalar_add(xd, xt, negmed_sb[:, k:k + 1])
                    axd = elem.tile([128, TBLK], _F32, tag="axd")
                    nc.scalar.activation(axd, xt, mybir.ActivationFunctionType.Abs,
                                         bias=negmed_sb[:, k:k + 1], scale=1.0)
                    xdb = elem.tile([128, TBLK], _BF16, tag="xdb")
                    nc.scalar.copy(xdb, xd)
                    masks = []
                    for n in range(NS_CORE):
                        mk = elem.tile([128, TBLK], _BF16, tag=f"mk{n}")
                        nc.vector.tensor_scalar(mk, axd, thr_sb[n][:, k:k + 1],
                                                None, op0=mybir.AluOpType.is_gt)
                        masks.append(mk)
                        xm = xmp.tile([128, TBLK], _BF16, tag=f"xm{n}_{k}")
                        nc.vector.tensor_tensor(out=xm, in0=xdb, in1=mk,
                                                op=mybir.AluOpType.mult)
                        xms[n][k] = xm
                    # mask-sum tree: per-chunk on GPSIMD, pair+quad on DVE
                    # (values <= 8, exact in bf16) -> 4 count matmuls per block
                    ms = elem.tile([128, TBLK], _BF16, tag="ms", bufs=3)
                    nc.gpsimd.tensor_tensor(out=ms, in0=masks[0], in1=masks[1],
                                            op=mybir.AluOpType.add)
                    if k % 2 == 0:
                        ms_prev = ms
                    else:
                        ms2 = elem.tile([128, TBLK], _BF16, tag="ms2", bufs=3)
                        nc.vector.tensor_tensor(out=ms2, in0=ms_prev, in1=ms,
                                                op=mybir.AluOpType.add)
                        if (k // 2) % 2 == 0:
                            ms2_prev = ms2
                        else:
                            ms4 = elem.tile([128, TBLK], _BF16, tag="ms4", bufs=3)
                            nc.vector.tensor_tensor(out=ms4, in0=ms2_prev, in1=ms2,
                                                    op=mybir.AluOpType.add)
                            nc.tensor.matmul(cps, ones_sb, ms4,
                                             start=(k == 3), stop=(k == KC - 1))
                csb = yout.tile([1, TBLK], _F32, tag="csb")
                nc.scalar.copy(csb, cps)
                nc.sync.dma_start(out=cnt[0:1, tsl], in_=csb)

                # y matmuls: y^T[o_chunk, t] accumulated over k
                for n in range(NS_CORE):
                    for o in range(STRIPE // 128):
                        col0 = n * STRIPE + o * 128
                        yps = ps.tile([128, TBLK], _F32, tag="yps")
                        for k in range(KC):
                            nc.tensor.matmul(
                                yps,
                                w_sb[k][:, col0:col0 + 128],
                                xms[n][k],
                                start=(k == 0), stop=(k == KC - 1))
                        ysb = yout.tile([128, TBLK], _F32, tag="ysb")
                        oc = col0 // 128
                        nc.scalar.activation(
                            ysb, yps, mybir.ActivationFunctionType.Identity,
                            bias=pmb_sb[:, oc:oc + 1], scale=1.0)
                        nc.sync.dma_start(out=yT[col0:col0 + 128, tsl], in_=ysb)
    nc.finalize()
    return nc


_NC_CACHE = None


def _get_nc():
    global _NC_CACHE
    if _NC_CACHE is None:
        _NC_CACHE = _build()
    return _NC_CACHE


def _prep_in_maps(x, weight, bias, thresholds, adj_med, adj_std):
    x = np.asarray(x, dtype=np.float32)
    weight = np.asarray(weight, dtype=np.float32)
    bias = np.asarray(bias, dtype=np.float32)
    thresholds = np.asarray(thresholds, dtype=np.float32)
    adj_med = np.asarray(adj_med, dtype=np.float32)
    adj_std = np.asarray(adj_std, dtype=np.float32)

    xT = np.ascontiguousarray(x.reshape(TOK, IN_F).T)
    # effective thresholds, matching the reference's op order exactly:
    # (thresholds * adj_std) * SCALE, all fp32 elementwise
    thr_eff = (thresholds * adj_std[None, :]) * THRESH_LR_SCALE
    negmed = np.ascontiguousarray((-adj_med).reshape(KC, 128))
    post_mu_bias = (adj_med @ weight + bias).astype(np.float32)

    in_maps = []
    for c in range(N_CORES):
        cols = slice(c * O_CORE, (c + 1) * O_CORE)
        wb = np.ascontiguousarray(weight[:, cols]).astype(ml_dtypes.bfloat16)
        thr_c = np.ascontiguousarray(
            thr_eff[c * NS_CORE:(c + 1) * NS_CORE].reshape(NS_CORE, KC, 128))
        pmb_c = np.ascontiguousarray(post_mu_bias[cols].reshape(O_CORE // 128, 128))
        in_maps.append({"xT": xT, "wb": wb, "negmed": negmed,
                        "thr": thr_c, "pmb": pmb_c})
    return in_maps


def _run(inputs, trace=False, trace_kwargs=None):
    nc = _get_nc()
    in_maps = _prep_in_maps(**inputs)
    kw = {}
    if trace:
        kw["trace"] = True
        if trace_kwargs:
            kw.update(trace_kwargs)
    return run_bass_kernel_spmd(nc, in_maps, core_ids=list(range(N_CORES)), **kw)


def _assemble(results):
    y = np.empty((TOK, OUT_F), dtype=np.float32)
    cnt_total = np.zeros(TOK, dtype=np.float32)
    for c, r in enumerate(results):
        y[:, c * O_CORE:(c + 1) * O_CORE] = r["yT"].T
        cnt_total += r["cnt"][0]
    y = y.reshape(1, TOK, OUT_F)
    active = (np.float32(STRIPE) * cnt_total).reshape(1, TOK)
    dense = np.full((1, TOK), np.float32(STRIPE * N_STRIPES * IN_F),
                    dtype=np.float32)
    return y, (dense, active)


def kernel(x, weight, bias, thresholds, adj_med, adj_std):
    res = _run(dict(x=x, weight=weight, bias=bias, thresholds=thresholds,
                    adj_med=adj_med, adj_std=adj_std))
    return _assemble(res.results)


# revision 18
# speedup vs baseline: 1.0079x; 1.0022x over previous
"""CWICLinear eval-mode forward on 8 Trainium2 NeuronCores.

Sharding: tensor-parallel over stripes — core c owns stripes {2c, 2c+1},
i.e. output columns [1024*c, 1024*(c+1)).  x is replicated (transposed on
host so the contraction dim lands on SBUF partitions); weight/thresholds/
bias are sliced per core.  Each core computes its y^T slice plus the
per-token active-feature counts for its two stripes; the host concatenates
y slices and sums the counts.

Numerics: the gate mask |x - med| > thr is computed bit-exactly in fp32
(GPSIMD/ACT subtract and Abs are exact, DVE is_gt is an exact fp32
compare), so active_params matches the reference exactly.  The matmul
runs in bf16 with fp32 PSUM accumulation (~3e-3 max rel err on y).
"""
import numpy as np
import ml_dtypes

import concourse.bacc as bacc_mod
import concourse.tile as tile
from concourse import mybir
from concourse.bass_utils import run_bass_kernel_spmd

# Problem constants (hardcoded per harness contract).
IN_F = 2048          # I: input features (contraction dim)
TOK = 2048           # T: batch*seq tokens
OUT_F = 8192         # total output features
STRIPE = 512
N_STRIPES = 16
N_CORES = 8
O_CORE = OUT_F // N_CORES        # 1024 output cols per core
NS_CORE = N_STRIPES // N_CORES   # 2 stripes per core
KC = IN_F // 128                 # 16 contraction chunks
TBLK = 512                       # token block per outer iteration
NTB = TOK // TBLK                # 4 token blocks
THRESH_LR_SCALE = np.float32(IN_F ** 0.5)

_BF16 = mybir.dt.bfloat16
_F32 = mybir.dt.float32


def _build():
    nc = bacc_mod.Bacc("TRN2")
    xT = nc.dram_tensor("xT", [IN_F, TOK], _F32, kind="ExternalInput")
    wb = nc.dram_tensor("wb", [IN_F, O_CORE], _BF16, kind="ExternalInput")
    negmed = nc.dram_tensor("negmed", [KC, 128], _F32, kind="ExternalInput")
    thr = nc.dram_tensor("thr", [NS_CORE, KC, 128], _F32, kind="ExternalInput")
    pmb = nc.dram_tensor("pmb", [O_CORE // 128, 128], _F32, kind="ExternalInput")

    yT = nc.dram_tensor("yT", [O_CORE, TOK], _F32, kind="ExternalOutput")
    cnt = nc.dram_tensor("cnt", [1, TOK], _F32, kind="ExternalOutput")

    with tile.TileContext(nc) as tc:
        with tc.tile_pool(name="const", bufs=1) as constp, \
             tc.tile_pool(name="wp", bufs=1) as wp, \
             tc.tile_pool(name="xin", bufs=3) as xin, \
             tc.tile_pool(name="elem", bufs=3) as elem, \
             tc.tile_pool(name="xmp", bufs=2) as xmp, \
             tc.tile_pool(name="yout", bufs=4) as yout, \
             tc.tile_pool(name="ps", bufs=7, space="PSUM") as ps, \
             tc.tile_pool(name="psc", bufs=1, space="PSUM") as psc:

            # ---- persistent constants ----
            negmed_sb = constp.tile([128, KC], _F32)
            nc.gpsimd.dma_start(out=negmed_sb,
                                in_=negmed[:, :].rearrange("k p -> p k"))
            thr_sb = []
            for n in range(NS_CORE):
                t = constp.tile([128, KC], _F32, tag=f"thr{n}")
                nc.gpsimd.dma_start(out=t, in_=thr[n].rearrange("k p -> p k"))
                thr_sb.append(t)
            pmb_sb = constp.tile([128, O_CORE // 128], _F32)
            nc.gpsimd.dma_start(out=pmb_sb,
                                in_=pmb[:, :].rearrange("o p -> p o"))
            ones_sb = constp.tile([128, 1], _BF16)
            nc.vector.memset(ones_sb, 1.0)

            # PE warm-up: ~4us of tiny matmuls during the otherwise-idle
            # head so the HAM clock gate reaches 8/8 before real work.
            # They write cnt psum [0,0], which the real count chain's
            # start=True overwrites.
            warm_cps = psc.tile([1, TBLK], _F32, tag="cps")
            for _ in range(140):
                nc.tensor.matmul(warm_cps[0:1, 0:1], ones_sb, ones_sb,
                                 start=True, stop=True)

            # ---- resident weights (bf16, cast on host), loaded interleaved
            # with the first token block's x tiles so PE starts early ----
            w_sb = [None] * KC

            for tb in range(NTB):
                tsl = slice(tb * TBLK, (tb + 1) * TBLK)
                xms = [[None] * KC for _ in range(NS_CORE)]
                cps = psc.tile([1, TBLK], _F32, tag="cps")
                for k in range(KC):
                    xt = xin.tile([128, TBLK], _F32, tag="xt")
                    nc.sync.dma_start(out=xt, in_=xT[k * 128:(k + 1) * 128, tsl])
                    if tb == 0:
                        t = wp.tile([128, O_CORE], _BF16, tag=f"w{k}")
                        nc.sync.dma_start(out=t, in_=wb[k * 128:(k + 1) * 128, :])
                        w_sb[k] = t
                    # exact xd = x - med on GPSIMD; exact |xd| on ACT
                    xd = elem.tile([128, TBLK], _F32, tag="xd")
                    nc.gpsimd.tensor_scalar_add(xd, xt, negmed_sb[:, k:k + 1])
                    axd = elem.tile([128, TBLK], _F32, tag="axd")
                    nc.scalar.activation(axd, xt, mybir.ActivationFunctionType.Abs,
                                         bias=negmed_sb[:, k:k + 1], scale=1.0)
                    xdb = elem.tile([128, TBLK], _BF16, tag="xdb")
                    nc.scalar.copy(xdb, xd)
                    masks = []
                    for n in range(NS_CORE):
                        mk = elem.tile([128, TBLK], _BF16, tag=f"mk{n}")
                        nc.vector.tensor_scalar(mk, axd, thr_sb[n][:, k:k + 1],
                                                None, op0=mybir.AluOpType.is_gt)
                        masks.append(mk)
                        xm = xmp.tile([128, TBLK], _BF16, tag=f"xm{n}_{k}")
                        nc.vector.tensor_tensor(out=xm, in0=xdb, in1=mk,
                                                op=mybir.AluOpType.mult)
                        xms[n][k] = xm
                    # mask-sum tree: per-chunk on GPSIMD, pair+quad on DVE
                    # (values <= 8, exact in bf16) -> 4 count matmuls per block
                    ms = elem.tile([128, TBLK], _BF16, tag="ms", bufs=3)
                    nc.gpsimd.tensor_tensor(out=ms, in0=masks[0], in1=masks[1],
                                            op=mybir.AluOpType.add)
                    if k % 2 == 0:
                        ms_prev = ms
                    else:
                        ms2 = elem.tile([128, TBLK], _BF16, tag="ms2", bufs=3)
                        nc.vector.tensor_tensor(out=ms2, in0=ms_prev, in1=ms,
                                                op=mybir.AluOpType.add)
                        if (k // 2) % 2 == 0:
                            ms2_prev = ms2
                        else:
                            ms4 = elem.tile([128, TBLK], _BF16, tag="ms4", bufs=3)
                            nc.vector.tensor_tensor(out=ms4, in0=ms2_prev, in1=ms2,
                                                    op=mybir.AluOpType.add)
                            nc.tensor.matmul(cps, ones_sb, ms4,
                                             start=(k == 3), stop=(k == KC - 1))
                csb = yout.tile([1, TBLK], _F32, tag="csb")
                nc.scalar.copy(csb, cps)
                nc.sync.dma_start(out=cnt[0:1, tsl], in_=csb)

                # y matmuls: y^T[o_chunk, t] accumulated over k
                for n in range(NS_CORE):
                    for o in range(STRIPE // 128):
                        col0 = n * STRIPE + o * 128
                        yps = ps.tile([128, TBLK], _F32, tag="yps")
                        for k in range(KC):
                            nc.tensor.matmul(
                                yps,
                                w_sb[k][:, col0:col0 + 128],
                                xms[n][k],
                                start=(k == 0), stop=(k == KC - 1))
                        ysb = yout.tile([128, TBLK], _F32, tag="ysb")
                        oc = col0 // 128
                        nc.scalar.activation(
                            ysb, yps, mybir.ActivationFunctionType.Identity,
                            bias=pmb_sb[:, oc:oc + 1], scale=1.0)
                        nc.sync.dma_start(out=yT[col0:col0 + 128, tsl], in_=ysb)
    nc.finalize()
    return nc


_NC_CACHE = None


def _get_nc():
    global _NC_CACHE
    if _NC_CACHE is None:
        _NC_CACHE = _build()
    return _NC_CACHE


def _prep_in_maps(x, weight, bias, thresholds, adj_med, adj_std):
    x = np.asarray(x, dtype=np.float32)
    weight = np.asarray(weight, dtype=np.float32)
    bias = np.asarray(bias, dtype=np.float32)
    thresholds = np.asarray(thresholds, dtype=np.float32)
    adj_med = np.asarray(adj_med, dtype=np.float32)
    adj_std = np.asarray(adj_std, dtype=np.float32)

    xT = np.ascontiguousarray(x.reshape(TOK, IN_F).T)
    # effective thresholds, matching the reference's op order exactly:
    # (thresholds * adj_std) * SCALE, all fp32 elementwise
    thr_eff = (thresholds * adj_std[None, :]) * THRESH_LR_SCALE
    negmed = np.ascontiguousarray((-adj_med).reshape(KC, 128))
    post_mu_bias = (adj_med @ weight + bias).astype(np.float32)

    in_maps = []
    for c in range(N_CORES):
        cols = slice(c * O_CORE, (c + 1) * O_CORE)
        wb = np.ascontiguousarray(weight[:, cols]).astype(ml_dtypes.bfloat16)
        thr_c = np.ascontiguousarray(
            thr_eff[c * NS_CORE:(c + 1) * NS_CORE].reshape(NS_CORE, KC, 128))
        pmb_c = np.ascontiguousarray(post_mu_bias[cols].reshape(O_CORE // 128, 128))
        in_maps.append({"xT": xT, "wb": wb, "negmed": negmed,
                        "thr": thr_c, "pmb": pmb_c})
    return in_maps


def _run(inputs, trace=False, trace_kwargs=None):
    nc = _get_nc()
    in_maps = _prep_in_maps(**inputs)
    kw = {}
    if trace:
        kw["trace"] = True
        if trace_kwargs:
            kw.update(trace_kwargs)
    return run_bass_kernel_spmd(nc, in_maps, core_ids=list(range(N_CORES)), **kw)


def _assemble(results):
    y = np.empty((TOK, OUT_F), dtype=np.float32)
    cnt_total = np.zeros(TOK, dtype=np.float32)
    for c, r in enumerate(results):
        y[:, c * O_CORE:(c + 1) * O_CORE] = r["yT"].T
        cnt_total += r["cnt"][0]
    y = y.reshape(1, TOK, OUT_F)
    active = (np.float32(STRIPE) * cnt_total).reshape(1, TOK)
    dense = np.full((1, TOK), np.float32(STRIPE * N_STRIPES * IN_F),
                    dtype=np.float32)
    return y, (dense, active)


def kernel(x, weight, bias, thresholds, adj_med, adj_std):
    res = _run(dict(x=x, weight=weight, bias=bias, thresholds=thresholds,
                    adj_med=adj_med, adj_std=adj_std))
    return _assemble(res.results)


# revision 27
# speedup vs baseline: 1.0156x; 1.0077x over previous
"""CWICLinear eval-mode forward on 8 Trainium2 NeuronCores.

Sharding: tensor-parallel over stripes — core c owns stripes {2c, 2c+1},
i.e. output columns [1024*c, 1024*(c+1)).  x is replicated (transposed on
host so the contraction dim lands on SBUF partitions); weight/thresholds/
bias are sliced per core.  Each core computes its y^T slice plus the
per-token active-feature counts for its two stripes; the host concatenates
y slices and sums the counts.

Numerics: the gate mask |x - med| > thr is computed bit-exactly in fp32
(GPSIMD/ACT subtract and Abs are exact, DVE is_gt is an exact fp32
compare), so active_params matches the reference exactly.  The matmul
runs in bf16 with fp32 PSUM accumulation (~3e-3 max rel err on y).
"""
import numpy as np
import ml_dtypes

import concourse.bacc as bacc_mod
import concourse.tile as tile
from concourse import mybir
from concourse.bass_utils import run_bass_kernel_spmd

# Problem constants (hardcoded per harness contract).
IN_F = 2048          # I: input features (contraction dim)
TOK = 2048           # T: batch*seq tokens
OUT_F = 8192         # total output features
STRIPE = 512
N_STRIPES = 16
N_CORES = 8
O_CORE = OUT_F // N_CORES        # 1024 output cols per core
NS_CORE = N_STRIPES // N_CORES   # 2 stripes per core
KC = IN_F // 128                 # 16 contraction chunks
TBLK = 512                       # token block per outer iteration
NTB = TOK // TBLK                # 4 token blocks
THRESH_LR_SCALE = np.float32(IN_F ** 0.5)

_BF16 = mybir.dt.bfloat16
_F32 = mybir.dt.float32


def _build():
    nc = bacc_mod.Bacc("TRN2")
    xT = nc.dram_tensor("xT", [IN_F, TOK], _F32, kind="ExternalInput")
    wb = nc.dram_tensor("wb", [IN_F, O_CORE], _BF16, kind="ExternalInput")
    negmed = nc.dram_tensor("negmed", [KC, 128], _F32, kind="ExternalInput")
    thr = nc.dram_tensor("thr", [NS_CORE, KC, 128], _F32, kind="ExternalInput")
    pmb = nc.dram_tensor("pmb", [O_CORE // 128, 128], _F32, kind="ExternalInput")

    yT = nc.dram_tensor("yT", [O_CORE, TOK], _F32, kind="ExternalOutput")
    cnt = nc.dram_tensor("cnt", [1, TOK], _F32, kind="ExternalOutput")

    with tile.TileContext(nc) as tc:
        with tc.tile_pool(name="const", bufs=1) as constp, \
             tc.tile_pool(name="wp", bufs=1) as wp, \
             tc.tile_pool(name="xin", bufs=3) as xin, \
             tc.tile_pool(name="elem", bufs=3) as elem, \
             tc.tile_pool(name="xmp", bufs=2) as xmp, \
             tc.tile_pool(name="yout", bufs=4) as yout, \
             tc.tile_pool(name="ps", bufs=7, space="PSUM") as ps, \
             tc.tile_pool(name="psc", bufs=1, space="PSUM") as psc:

            # first x tile issued ahead of everything else on the HWDGE queue
            xt_first = xin.tile([128, TBLK], _F32, tag="xt")
            nc.sync.dma_start(out=xt_first, in_=xT[0:128, 0:TBLK])

            # ---- persistent constants (fast HWDGE queue; pmb is only
            # needed ~20us in, so it rides the slow SWDGE queue) ----
            negmed_sb = constp.tile([128, KC], _F32)
            nc.sync.dma_start(out=negmed_sb,
                              in_=negmed[:, :].rearrange("k p -> p k"))
            thr_sb = []
            for n in range(NS_CORE):
                t = constp.tile([128, KC], _F32, tag=f"thr{n}")
                nc.gpsimd.dma_start(out=t, in_=thr[n].rearrange("k p -> p k"))
                thr_sb.append(t)
            pmb_sb = constp.tile([128, O_CORE // 128], _F32)
            nc.gpsimd.dma_start(out=pmb_sb,
                                in_=pmb[:, :].rearrange("o p -> p o"))
            ones_sb = constp.tile([128, 1], _BF16)
            nc.vector.memset(ones_sb, 1.0)

            # Trigger ACT's lazy LoadActFuncSet (~1.3us) at t=0, off the
            # critical path of the first real Abs.
            act_junk = constp.tile([128, 1], _F32)
            nc.vector.memset(act_junk, 0.0)
            act_junk2 = constp.tile([128, 1], _F32)
            nc.scalar.activation(act_junk2, act_junk,
                                 mybir.ActivationFunctionType.Abs)

            # PE warm-up: ~4.5us of N=512 matmuls on a junk tile during the
            # otherwise-idle head, so the HAM clock gate reaches 8/8 and the
            # pipeline is hot when real work arrives.  They write the cnt
            # psum, which the real count chain's start=True overwrites.
            warm_rhs = constp.tile([128, TBLK], _BF16)
            nc.vector.memset(warm_rhs, 0.0)
            warm_cps = psc.tile([1, TBLK], _F32, tag="cps")
            for _ in range(21):
                nc.tensor.matmul(warm_cps, ones_sb, warm_rhs,
                                 start=True, stop=True)

            # ---- resident weights (bf16, cast on host), loaded interleaved
            # with the first token block's x tiles so PE starts early ----
            w_sb = [None] * KC

            for tb in range(NTB):
                tsl = slice(tb * TBLK, (tb + 1) * TBLK)
                xms = [[None] * KC for _ in range(NS_CORE)]
                cps = psc.tile([1, TBLK], _F32, tag="cps")
                for k in range(KC):
                    if tb == 0 and k == 0:
                        xt = xt_first
                    else:
                        xt = xin.tile([128, TBLK], _F32, tag="xt")
                        nc.sync.dma_start(out=xt,
                                          in_=xT[k * 128:(k + 1) * 128, tsl])
                    if tb == 0:
                        t = wp.tile([128, O_CORE], _BF16, tag=f"w{k}")
                        nc.sync.dma_start(out=t, in_=wb[k * 128:(k + 1) * 128, :])
                        w_sb[k] = t
                    # exact xd = x - med on GPSIMD; exact |xd| on ACT
                    xd = elem.tile([128, TBLK], _F32, tag="xd")
                    nc.gpsimd.tensor_scalar_add(xd, xt, negmed_sb[:, k:k + 1])
                    axd = elem.tile([128, TBLK], _F32, tag="axd")
                    nc.scalar.activation(axd, xt, mybir.ActivationFunctionType.Abs,
                                         bias=negmed_sb[:, k:k + 1], scale=1.0)
                    xdb = elem.tile([128, TBLK], _BF16, tag="xdb")
                    nc.scalar.copy(xdb, xd)
                    masks = []
                    for n in range(NS_CORE):
                        mk = elem.tile([128, TBLK], _BF16, tag=f"mk{n}")
                        nc.vector.tensor_scalar(mk, axd, thr_sb[n][:, k:k + 1],
                                                None, op0=mybir.AluOpType.is_gt)
                        masks.append(mk)
                        xm = xmp.tile([128, TBLK], _BF16, tag=f"xm{n}_{k}")
                        nc.vector.tensor_tensor(out=xm, in0=xdb, in1=mk,
                                                op=mybir.AluOpType.mult)
                        xms[n][k] = xm
                    # mask-sum tree: per-chunk on GPSIMD, pair+quad on DVE
                    # (values <= 8, exact in bf16) -> 4 count matmuls per block
                    ms = elem.tile([128, TBLK], _BF16, tag="ms", bufs=3)
                    nc.gpsimd.tensor_tensor(out=ms, in0=masks[0], in1=masks[1],
                                            op=mybir.AluOpType.add)
                    if k % 2 == 0:
                        ms_prev = ms
                    else:
                        ms2 = elem.tile([128, TBLK], _BF16, tag="ms2", bufs=3)
                        nc.vector.tensor_tensor(out=ms2, in0=ms_prev, in1=ms,
                                                op=mybir.AluOpType.add)
                        if (k // 2) % 2 == 0:
                            ms2_prev = ms2
                        else:
                            ms4 = elem.tile([128, TBLK], _BF16, tag="ms4", bufs=3)
                            nc.vector.tensor_tensor(out=ms4, in0=ms2_prev, in1=ms2,
                                                    op=mybir.AluOpType.add)
                            nc.tensor.matmul(cps, ones_sb, ms4,
                                             start=(k == 3), stop=(k == KC - 1))
                csb = yout.tile([1, TBLK], _F32, tag="csb")
                nc.scalar.copy(csb, cps)
                nc.sync.dma_start(out=cnt[0:1, tsl], in_=csb)

                # y matmuls: y^T[o_chunk, t] accumulated over k
                for n in range(NS_CORE):
                    for o in range(STRIPE // 128):
                        col0 = n * STRIPE + o * 128
                        yps = ps.tile([128, TBLK], _F32, tag="yps")
                        for k in range(KC):
                            nc.tensor.matmul(
                                yps,
                                w_sb[k][:, col0:col0 + 128],
                                xms[n][k],
                                start=(k == 0), stop=(k == KC - 1))
                        ysb = yout.tile([128, TBLK], _F32, tag="ysb")
                        oc = col0 // 128
                        nc.scalar.activation(
                            ysb, yps, mybir.ActivationFunctionType.Identity,
                            bias=pmb_sb[:, oc:oc + 1], scale=1.0)
                        nc.sync.dma_start(out=yT[col0:col0 + 128, tsl], in_=ysb)
    nc.finalize()
    return nc


_NC_CACHE = None


def _get_nc():
    global _NC_CACHE
    if _NC_CACHE is None:
        _NC_CACHE = _build()
    return _NC_CACHE


def _prep_in_maps(x, weight, bias, thresholds, adj_med, adj_std):
    x = np.asarray(x, dtype=np.float32)
    weight = np.asarray(weight, dtype=np.float32)
    bias = np.asarray(bias, dtype=np.float32)
    thresholds = np.asarray(thresholds, dtype=np.float32)
    adj_med = np.asarray(adj_med, dtype=np.float32)
    adj_std = np.asarray(adj_std, dtype=np.float32)

    xT = np.ascontiguousarray(x.reshape(TOK, IN_F).T)
    # effective thresholds, matching the reference's op order exactly:
    # (thresholds * adj_std) * SCALE, all fp32 elementwise
    thr_eff = (thresholds * adj_std[None, :]) * THRESH_LR_SCALE
    negmed = np.ascontiguousarray((-adj_med).reshape(KC, 128))
    post_mu_bias = (adj_med @ weight + bias).astype(np.float32)

    in_maps = []
    for c in range(N_CORES):
        cols = slice(c * O_CORE, (c + 1) * O_CORE)
        wb = np.ascontiguousarray(weight[:, cols]).astype(ml_dtypes.bfloat16)
        thr_c = np.ascontiguousarray(
            thr_eff[c * NS_CORE:(c + 1) * NS_CORE].reshape(NS_CORE, KC, 128))
        pmb_c = np.ascontiguousarray(post_mu_bias[cols].reshape(O_CORE // 128, 128))
        in_maps.append({"xT": xT, "wb": wb, "negmed": negmed,
                        "thr": thr_c, "pmb": pmb_c})
    return in_maps


def _run(inputs, trace=False, trace_kwargs=None):
    nc = _get_nc()
    in_maps = _prep_in_maps(**inputs)
    kw = {}
    if trace:
        kw["trace"] = True
        if trace_kwargs:
            kw.update(trace_kwargs)
    return run_bass_kernel_spmd(nc, in_maps, core_ids=list(range(N_CORES)), **kw)


def _assemble(results):
    y = np.empty((TOK, OUT_F), dtype=np.float32)
    cnt_total = np.zeros(TOK, dtype=np.float32)
    for c, r in enumerate(results):
        y[:, c * O_CORE:(c + 1) * O_CORE] = r["yT"].T
        cnt_total += r["cnt"][0]
    y = y.reshape(1, TOK, OUT_F)
    active = (np.float32(STRIPE) * cnt_total).reshape(1, TOK)
    dense = np.full((1, TOK), np.float32(STRIPE * N_STRIPES * IN_F),
                    dtype=np.float32)
    return y, (dense, active)


def kernel(x, weight, bias, thresholds, adj_med, adj_std):
    res = _run(dict(x=x, weight=weight, bias=bias, thresholds=thresholds,
                    adj_med=adj_med, adj_std=adj_std))
    return _assemble(res.results)


# revision 38
# speedup vs baseline: 1.0235x; 1.0078x over previous
"""CWICLinear eval-mode forward on 8 Trainium2 NeuronCores.

Sharding: tensor-parallel over stripes — core c owns stripes {2c, 2c+1},
i.e. output columns [1024*c, 1024*(c+1)).  x is replicated (transposed on
host so the contraction dim lands on SBUF partitions); weight/thresholds/
bias are sliced per core.  Each core computes its y^T slice plus the
per-token active-feature counts for its two stripes; the host concatenates
y slices and sums the counts.

Numerics: the gate mask |x - med| > thr is computed bit-exactly in fp32
(GPSIMD/ACT subtract and Abs are exact, DVE is_gt is an exact fp32
compare), so active_params matches the reference exactly.  The matmul
runs in bf16 with fp32 PSUM accumulation (~3e-3 max rel err on y).
"""
import numpy as np
import ml_dtypes

import concourse.bacc as bacc_mod
import concourse.tile as tile
from concourse import mybir
from concourse.bass_utils import run_bass_kernel_spmd

# Problem constants (hardcoded per harness contract).
IN_F = 2048          # I: input features (contraction dim)
TOK = 2048           # T: batch*seq tokens
OUT_F = 8192         # total output features
STRIPE = 512
N_STRIPES = 16
N_CORES = 8
O_CORE = OUT_F // N_CORES        # 1024 output cols per core
NS_CORE = N_STRIPES // N_CORES   # 2 stripes per core
KC = IN_F // 128                 # 16 contraction chunks
TBLK = 512                       # token block per outer iteration
NTB = TOK // TBLK                # 4 token blocks
THRESH_LR_SCALE = np.float32(IN_F ** 0.5)

_BF16 = mybir.dt.bfloat16
_F32 = mybir.dt.float32


def _build():
    nc = bacc_mod.Bacc("TRN2")
    xT = nc.dram_tensor("xT", [IN_F, TOK], _F32, kind="ExternalInput")
    wb = nc.dram_tensor("wb", [IN_F, O_CORE], _BF16, kind="ExternalInput")
    negmed = nc.dram_tensor("negmed", [KC, 128], _F32, kind="ExternalInput")
    thr = nc.dram_tensor("thr", [NS_CORE, KC, 128], _F32, kind="ExternalInput")
    pmb = nc.dram_tensor("pmb", [O_CORE // 128, 128], _F32, kind="ExternalInput")

    yT = nc.dram_tensor("yT", [O_CORE, TOK], _F32, kind="ExternalOutput")
    cnt = nc.dram_tensor("cnt", [1, TOK], _F32, kind="ExternalOutput")

    with tile.TileContext(nc) as tc:
        with tc.tile_pool(name="const", bufs=1) as constp, \
             tc.tile_pool(name="wp", bufs=1) as wp, \
             tc.tile_pool(name="xin", bufs=3) as xin, \
             tc.tile_pool(name="elem", bufs=3) as elem, \
             tc.tile_pool(name="xmp", bufs=2) as xmp, \
             tc.tile_pool(name="yout", bufs=4) as yout, \
             tc.tile_pool(name="ps", bufs=7, space="PSUM") as ps, \
             tc.tile_pool(name="psc", bufs=1, space="PSUM") as psc:

            # first x tile issued ahead of everything else on the HWDGE queue
            xt_first = xin.tile([128, TBLK], _F32, tag="xt")
            nc.sync.dma_start(out=xt_first, in_=xT[0:128, 0:TBLK])

            # ---- persistent constants (fast HWDGE queue; pmb is only
            # needed ~20us in, so it rides the slow SWDGE queue) ----
            negmed_sb = constp.tile([128, KC], _F32)
            nc.sync.dma_start(out=negmed_sb,
                              in_=negmed[:, :].rearrange("k p -> p k"))
            thr_sb = []
            for n in range(NS_CORE):
                t = constp.tile([128, KC], _F32, tag=f"thr{n}")
                nc.scalar.dma_start(out=t, in_=thr[n].rearrange("k p -> p k"))
                thr_sb.append(t)
            pmb_sb = constp.tile([128, O_CORE // 128], _F32)
            nc.gpsimd.dma_start(out=pmb_sb,
                                in_=pmb[:, :].rearrange("o p -> p o"))
            ones_sb = constp.tile([128, 1], _BF16)
            nc.vector.memset(ones_sb, 1.0)

            # Trigger ACT's lazy LoadActFuncSet (~1.3us) at t=0, off the
            # critical path of the first real Abs.
            act_junk = constp.tile([128, 1], _F32)
            nc.vector.memset(act_junk, 0.0)
            act_junk2 = constp.tile([128, 1], _F32)
            nc.scalar.activation(act_junk2, act_junk,
                                 mybir.ActivationFunctionType.Abs)

            # PE warm-up: ~4.5us of N=512 matmuls on a junk tile during the
            # otherwise-idle head, so the HAM clock gate reaches 8/8 and the
            # pipeline is hot when real work arrives.  They write the cnt
            # psum, which the real count chain's start=True overwrites.
            warm_rhs = constp.tile([128, TBLK], _BF16)
            nc.vector.memset(warm_rhs, 0.0)
            warm_cps = psc.tile([1, TBLK], _F32, tag="cps")
            for _ in range(15):
                nc.tensor.matmul(warm_cps, ones_sb, warm_rhs,
                                 start=True, stop=True)

            # ---- resident weights (bf16, cast on host), loaded interleaved
            # with the first token block's x tiles so PE starts early ----
            w_sb = [None] * KC

            for tb in range(NTB):
                tsl = slice(tb * TBLK, (tb + 1) * TBLK)
                xms = [[None] * KC for _ in range(NS_CORE)]
                cps = psc.tile([1, TBLK], _F32, tag="cps")
                for k in range(KC):
                    if tb == 0 and k == 0:
                        xt = xt_first
                    else:
                        xt = xin.tile([128, TBLK], _F32, tag="xt")
                        nc.sync.dma_start(out=xt,
                                          in_=xT[k * 128:(k + 1) * 128, tsl])
                    if tb == 0:
                        t = wp.tile([128, O_CORE], _BF16, tag=f"w{k}")
                        nc.sync.dma_start(out=t, in_=wb[k * 128:(k + 1) * 128, :])
                        w_sb[k] = t
                    # exact xd = x - med on GPSIMD; exact |xd| on ACT
                    xd = elem.tile([128, TBLK], _F32, tag="xd")
                    nc.gpsimd.tensor_scalar_add(xd, xt, negmed_sb[:, k:k + 1])
                    axd = elem.tile([128, TBLK], _F32, tag="axd")
                    nc.scalar.activation(axd, xt, mybir.ActivationFunctionType.Abs,
                                         bias=negmed_sb[:, k:k + 1], scale=1.0)
                    xdb = elem.tile([128, TBLK], _BF16, tag="xdb")
                    nc.scalar.copy(xdb, xd)
                    masks = []
                    for n in range(NS_CORE):
                        mk = elem.tile([128, TBLK], _BF16, tag=f"mk{n}")
                        nc.vector.tensor_scalar(mk, axd, thr_sb[n][:, k:k + 1],
                                                None, op0=mybir.AluOpType.is_gt)
                        masks.append(mk)
                        xm = xmp.tile([128, TBLK], _BF16, tag=f"xm{n}_{k}")
                        nc.vector.tensor_tensor(out=xm, in0=xdb, in1=mk,
                                                op=mybir.AluOpType.mult)
                        xms[n][k] = xm
                    # mask-sum tree: per-chunk on GPSIMD, pair+quad on DVE
                    # (values <= 8, exact in bf16) -> 4 count matmuls per block
                    ms = elem.tile([128, TBLK], _BF16, tag="ms", bufs=3)
                    nc.gpsimd.tensor_tensor(out=ms, in0=masks[0], in1=masks[1],
                                            op=mybir.AluOpType.add)
                    if k % 2 == 0:
                        ms_prev = ms
                    else:
                        ms2 = elem.tile([128, TBLK], _BF16, tag="ms2", bufs=3)
                        nc.vector.tensor_tensor(out=ms2, in0=ms_prev, in1=ms,
                                                op=mybir.AluOpType.add)
                        if (k // 2) % 2 == 0:
                            ms2_prev = ms2
                        else:
                            ms4 = elem.tile([128, TBLK], _BF16, tag="ms4", bufs=3)
                            nc.vector.tensor_tensor(out=ms4, in0=ms2_prev, in1=ms2,
                                                    op=mybir.AluOpType.add)
                            if (k // 4) % 2 == 0:
                                ms4_prev = ms4
                            else:
                                ms8 = elem.tile([128, TBLK], _BF16, tag="ms8",
                                                bufs=3)
                                nc.vector.tensor_tensor(
                                    out=ms8, in0=ms4_prev, in1=ms4,
                                    op=mybir.AluOpType.add)
                                nc.tensor.matmul(cps, ones_sb, ms8,
                                                 start=(k == 7),
                                                 stop=(k == KC - 1))
                csb = yout.tile([1, TBLK], _F32, tag="csb")
                nc.scalar.copy(csb, cps)
                nc.sync.dma_start(out=cnt[0:1, tsl], in_=csb)

                # y matmuls: y^T[o_chunk, t] accumulated over k
                for n in range(NS_CORE):
                    for o in range(STRIPE // 128):
                        col0 = n * STRIPE + o * 128
                        yps = ps.tile([128, TBLK], _F32, tag="yps")
                        for k in range(KC):
                            nc.tensor.matmul(
                                yps,
                                w_sb[k][:, col0:col0 + 128],
                                xms[n][k],
                                start=(k == 0), stop=(k == KC - 1))
                        ysb = yout.tile([128, TBLK], _F32, tag="ysb")
                        oc = col0 // 128
                        nc.scalar.activation(
                            ysb, yps, mybir.ActivationFunctionType.Identity,
                            bias=pmb_sb[:, oc:oc + 1], scale=1.0)
                        nc.sync.dma_start(out=yT[col0:col0 + 128, tsl], in_=ysb)
    nc.finalize()
    return nc


_NC_CACHE = None


def _get_nc():
    global _NC_CACHE
    if _NC_CACHE is None:
        _NC_CACHE = _build()
    return _NC_CACHE


def _prep_in_maps(x, weight, bias, thresholds, adj_med, adj_std):
    x = np.asarray(x, dtype=np.float32)
    weight = np.asarray(weight, dtype=np.float32)
    bias = np.asarray(bias, dtype=np.float32)
    thresholds = np.asarray(thresholds, dtype=np.float32)
    adj_med = np.asarray(adj_med, dtype=np.float32)
    adj_std = np.asarray(adj_std, dtype=np.float32)

    xT = np.ascontiguousarray(x.reshape(TOK, IN_F).T)
    # effective thresholds, matching the reference's op order exactly:
    # (thresholds * adj_std) * SCALE, all fp32 elementwise
    thr_eff = (thresholds * adj_std[None, :]) * THRESH_LR_SCALE
    negmed = np.ascontiguousarray((-adj_med).reshape(KC, 128))
    post_mu_bias = (adj_med @ weight + bias).astype(np.float32)

    in_maps = []
    for c in range(N_CORES):
        cols = slice(c * O_CORE, (c + 1) * O_CORE)
        wb = np.ascontiguousarray(weight[:, cols]).astype(ml_dtypes.bfloat16)
        thr_c = np.ascontiguousarray(
            thr_eff[c * NS_CORE:(c + 1) * NS_CORE].reshape(NS_CORE, KC, 128))
        pmb_c = np.ascontiguousarray(post_mu_bias[cols].reshape(O_CORE // 128, 128))
        in_maps.append({"xT": xT, "wb": wb, "negmed": negmed,
                        "thr": thr_c, "pmb": pmb_c})
    return in_maps


def _run(inputs, trace=False, trace_kwargs=None):
    nc = _get_nc()
    in_maps = _prep_in_maps(**inputs)
    kw = {}
    if trace:
        kw["trace"] = True
        if trace_kwargs:
            kw.update(trace_kwargs)
    return run_bass_kernel_spmd(nc, in_maps, core_ids=list(range(N_CORES)), **kw)


def _assemble(results):
    y = np.empty((TOK, OUT_F), dtype=np.float32)
    cnt_total = np.zeros(TOK, dtype=np.float32)
    for c, r in enumerate(results):
        y[:, c * O_CORE:(c + 1) * O_CORE] = r["yT"].T
        cnt_total += r["cnt"][0]
    y = y.reshape(1, TOK, OUT_F)
    active = (np.float32(STRIPE) * cnt_total).reshape(1, TOK)
    dense = np.full((1, TOK), np.float32(STRIPE * N_STRIPES * IN_F),
                    dtype=np.float32)
    return y, (dense, active)


def kernel(x, weight, bias, thresholds, adj_med, adj_std):
    res = _run(dict(x=x, weight=weight, bias=bias, thresholds=thresholds,
                    adj_med=adj_med, adj_std=adj_std))
    return _assemble(res.results)
